# revision 17
# baseline (speedup 1.0000x reference)
"""Complex grouped MLP (nn_ComplexGroupedMLP) Trainium2 Bass kernel.

Math: y[b,m,g,o] = sum_k (x_re + i*x_im)[b,m,g,k] * (w_re + i*w_im)[g,k,o]
Shapes: x_* [4, 4096, 32, 64] f32, weight [32, 64, 64, 2] f32,
out [4, 4096, 32, 64, 2] f32.

Strategy (8 NeuronCores, data-parallel over tokens):
- Tokens = B*M = 16384, sharded 2048/core (M sliced into 8 chunks of 512).
- Complex arithmetic folded into ONE real matmul per (group, token-block):
    lhsT = W_cat[g] = [[Wre, Wim], [-Wim, Wre]]  (128 x 128, host-packed)
    rhs  = X^T_cat  = [x_re^T ; x_im^T]          (128 x 512 tokens)
    out  = [Yre^T ; Yim^T]                       (128 x 512) in PSUM
- x arrives token-major; the k-on-partitions layout is produced ON DEVICE
  with PE transpose-mode matmuls (fp32), PSUM -> SBUF via DVE.
- Output stored feature-major [g, (2*64), tokens]; host unpermutes.
"""

import os
import numpy as np

import concourse.bass as bass
import concourse.bacc as bacc
import concourse.mybir as mybir
from concourse.bass import ds, ts
from concourse.tile import TileContext
from concourse.bass_utils import run_bass_kernel_spmd
from concourse.masks import make_identity

# ---- hardcoded problem geometry (per spec) ----
B, M, G, K = 4, 4096, 32, 64
NCORES = 8
TOK = (B * M) // NCORES      # 2048 tokens per core
FEAT = G * K                 # 2048 features per token
P = 128                      # partitions
BLK = 512                    # tokens per matmul (max fp32 free dim / PSUM bank)
SUB = BLK // P               # 4 transposes per (group, block)
FP32 = mybir.dt.float32

_prog_cache = {}


def build_program(tok=TOK):
    """One-core SPMD program; identical for all 8 cores.

    xin is host-interleaved: [tok, G, (64 re | 64 im)] -> [tok, 2*FEAT/2]
    so each group's 128 k-features are contiguous per token row (walrus
    requires single-free-dim APs on matmul operands).
    """
    nblk = tok // BLK
    nc = bacc.Bacc(trn_type="TRN2")
    xin = nc.declare_dram_parameter("xin", [tok, G * 2 * K], FP32, isOutput=False)
    wc = nc.declare_dram_parameter("wc", [G, P, P], FP32, isOutput=False)
    yt = nc.declare_dram_parameter("yt", [G, P, tok], FP32, isOutput=True)

    with TileContext(nc) as tc:
        with (
            tc.tile_pool(name="const", bufs=1) as const_pool,
            tc.tile_pool(name="xinp", bufs=2) as xin_pool,
            tc.tile_pool(name="xt", bufs=3) as xt_pool,
            tc.tile_pool(name="yout", bufs=3) as y_pool,
            tc.tile_pool(name="pt", bufs=4, space="PSUM") as pt_pool,
            tc.tile_pool(name="py", bufs=2, space="PSUM") as py_pool,
        ):
            ident = const_pool.tile([P, P], FP32)
            make_identity(nc, ident)
            # W_cat in SBUF: [k_cat(128 part), g, m_cat(128)]
            wc_sb = const_pool.tile([P, G, P], FP32)
            nc.sync.dma_start(out=wc_sb[:], in_=wc[:].rearrange("g k m -> k g m"))

            # Walrus allows only ONE sync-wait on a (fp32 self-loading)
            # matmul. These prologue transposes absorb the ident(Pool) and
            # wc_sb(DMA) waits so steady-state matmuls each carry <=1 wait.
            warm0 = pt_pool.tile([P, P], FP32, tag="pt")
            nc.tensor.transpose(warm0[:], ident[:], ident[:])
            warm1 = pt_pool.tile([P, P], FP32, tag="pt")
            nc.tensor.transpose(warm1[:], wc_sb[:, 0, :], ident[:])

            last_xt = None
            for tb in range(nblk):
                # x block: [128 tok partitions, SUB, G*128 feats] via ONE DMA
                x_sb = xin_pool.tile([P, SUB, G * P], FP32, tag="x_sb")
                nc.sync.dma_start(
                    out=x_sb[:],
                    in_=xin[ds(tb * BLK, BLK), :].rearrange(
                        "(j p) f -> p j f", p=P
                    ),
                )
                for g in range(G):
                    # gather X^T_cat for this group: [128 (kre|kim), 512 tok]
                    xt_cat = xt_pool.tile([P, BLK], FP32, tag="xt_cat")
                    for j in range(SUB):
                        pt = pt_pool.tile([P, P], FP32, tag="pt")
                        # in_: [128 tok, 128 (kre|kim)] -> out: [128, 128 tok]
                        nc.tensor.transpose(
                            pt[:], x_sb[:, j, ds(P * g, P)], ident[:]
                        )
                        nc.vector.tensor_copy(out=xt_cat[:, ts(j, P)], in_=pt[:])
                    py = py_pool.tile([P, BLK], FP32, tag="py")
                    nc.tensor.matmul(
                        py[:], wc_sb[:, g, :], xt_cat[:], start=True, stop=True
                    )
                    y_sb = y_pool.tile([P, BLK], FP32, tag="y_sb")
                    nc.vector.tensor_copy(out=y_sb[:], in_=py[:])
                    # stores ride the ACT HWDGE ring so a compute-blocked
                    # store can't head-of-line-block the next input load
                    # (which rides the SP ring)
                    nc.scalar.dma_start(
                        out=yt[g, :, ds(tb * BLK, BLK)], in_=y_sb[:]
                    )
    nc.compile()
    return nc


def _get_program(tok=TOK):
    if tok not in _prog_cache:
        _prog_cache[tok] = build_program(tok)
    return _prog_cache[tok]


def _host_pack_w(weight):
    wre = np.ascontiguousarray(weight[..., 0], dtype=np.float32)  # [G,K,K]
    wim = np.ascontiguousarray(weight[..., 1], dtype=np.float32)
    wcat = np.empty((G, P, P), dtype=np.float32)
    wcat[:, :K, :K] = wre
    wcat[:, :K, K:] = wim
    wcat[:, K:, :K] = -wim
    wcat[:, K:, K:] = wre
    return wcat


# test.py can read the most recent BassKernelResults (exec_time_ns etc.)
last_results = None
last_trace_dir = None


def _install_ntff_hook():
    """The agent image's antenv lacks axon_hooks; register the NTFF profile
    hook ourselves so run_bass_kernel_spmd(trace=True) can time the kernel."""
    import sys
    import types

    if "antenv.axon_hooks" in sys.modules:
        return
    from trn_agent_boot.trn_boot import _ntff_profile_via_ctypes

    hook = _ntff_profile_via_ctypes("/opt/axon/libaxon_pjrt.so")
    mod = types.ModuleType("antenv.axon_hooks")
    mod.get_axon_ntff_profile_hook = lambda: hook
    sys.modules["antenv.axon_hooks"] = mod
    import antenv

    antenv.axon_hooks = mod
    import concourse.bass_utils as bu

    bu.upload_artifacts = lambda tmpdir: f"local://{tmpdir}"


def kernel(x_real, x_imag, weight):
    global last_results, last_trace_dir
    nc = _get_program()
    wcat = _host_pack_w(weight)
    msl = M // NCORES  # 512
    in_maps = []
    for c in range(NCORES):
        # interleave re/im per group: [tok, G, 2, K] contiguous
        xin_c = np.empty((TOK, G, 2, K), dtype=np.float32)
        xin_c[:, :, 0, :] = x_real[:, c * msl:(c + 1) * msl].reshape(TOK, G, K)
        xin_c[:, :, 1, :] = x_imag[:, c * msl:(c + 1) * msl].reshape(TOK, G, K)
        in_maps.append({"xin": xin_c.reshape(TOK, G * 2 * K), "wc": wcat})

    trace = bool(int(os.environ.get("KERNEL_TRACE", "0")))
    tmpdir = None
    if trace:
        import tempfile

        _install_ntff_hook()
        tmpdir = os.environ.get("KERNEL_TRACE_DIR") or tempfile.mkdtemp(
            prefix="bass_trace_"
        )
        last_trace_dir = tmpdir
    res = run_bass_kernel_spmd(
        nc, in_maps, list(range(NCORES)), trace=trace, tmpdir=tmpdir
    )
    last_results = res

    yt_all = np.stack([res.results[c]["yt"] for c in range(NCORES)])
    # [8, 32, 2(comp), 64(o), 4(b), 512(ml)] -> [b, c, ml, g, o, comp]
    y = (
        yt_all.reshape(NCORES, G, 2, K, B, msl)
        .transpose(4, 0, 5, 1, 3, 2)
        .reshape(B, M, G, K, 2)
    )
    return np.ascontiguousarray(y)


# revision 20
# speedup vs baseline: 1.3943x; 1.3943x over previous
"""Complex grouped MLP (nn_ComplexGroupedMLP) Trainium2 Bass kernel.

Math: y[b,m,g,o] = sum_k (x_re + i*x_im)[b,m,g,k] * (w_re + i*w_im)[g,k,o]
Shapes: x_* [4, 4096, 32, 64] f32, weight [32, 64, 64, 2] f32,
out [4, 4096, 32, 64, 2] f32.

Strategy (8 NeuronCores, data-parallel over tokens):
- Tokens = B*M = 16384, sharded 2048/core (M sliced into 8 chunks of 512).
- Complex arithmetic folded into ONE real matmul per (group, token-block):
    lhsT = W_cat[g] = [[Wre, Wim], [-Wim, Wre]]  (128 x 128, host-packed)
    rhs  = X^T_cat  = [x_re^T ; x_im^T]          (128 x 512 tokens)
    out  = [Yre^T ; Yim^T]                       (128 x 512) in PSUM
- x arrives token-major; the k-on-partitions layout is produced ON DEVICE
  with PE transpose-mode matmuls (fp32), PSUM -> SBUF via DVE.
- Output stored feature-major [g, (2*64), tokens]; host unpermutes.
"""

import os
import numpy as np

import concourse.bass as bass
import concourse.bacc as bacc
import concourse.mybir as mybir
from concourse.bass import ds, ts
from concourse.tile import TileContext
from concourse.bass_utils import run_bass_kernel_spmd
from concourse.masks import make_identity

# ---- hardcoded problem geometry (per spec) ----
B, M, G, K = 4, 4096, 32, 64
NCORES = 8
TOK = (B * M) // NCORES      # 2048 tokens per core
FEAT = G * K                 # 2048 features per token
P = 128                      # partitions
BLK = 512                    # tokens per matmul (max fp32 free dim / PSUM bank)
SUB = BLK // P               # 4 transposes per (group, block)
FP32 = mybir.dt.float32

_prog_cache = {}


def build_program(tok=TOK, f32r=False):
    """One-core SPMD program; identical for all 8 cores.

    xin is host-interleaved: [tok, G, (64 re | 64 im)] -> [tok, 2*FEAT/2]
    so each group's 128 k-features are contiguous per token row (walrus
    requires single-free-dim APs on matmul operands).
    """
    nblk = tok // BLK
    nc = bacc.Bacc(trn_type="TRN2")
    F32R = mybir.dt.float32r
    xin = nc.declare_dram_parameter("xin", [tok, G * 2 * K], FP32, isOutput=False)
    wc = nc.declare_dram_parameter("wc", [G, P, P], FP32, isOutput=False)
    yt = nc.declare_dram_parameter("yt", [G, P, tok], FP32, isOutput=True)

    with TileContext(nc) as tc:
        with (
            tc.tile_pool(name="const", bufs=1) as const_pool,
            tc.tile_pool(name="xinp", bufs=2) as xin_pool,
            tc.tile_pool(name="xt", bufs=3) as xt_pool,
            tc.tile_pool(name="yout", bufs=3) as y_pool,
            tc.tile_pool(name="pt", bufs=3, space="PSUM") as pt_pool,
            tc.tile_pool(name="py", bufs=3, space="PSUM") as py_pool,
        ):
            ident = const_pool.tile([P, P], FP32)
            make_identity(nc, ident)
            # W_cat in SBUF: [k_cat(128 part), g, m_cat(128)]
            wc_sb = const_pool.tile([P, G, P], FP32)
            nc.sync.dma_start(out=wc_sb[:], in_=wc[:].rearrange("g k m -> k g m"))

            # Walrus allows only ONE sync-wait on a (fp32 self-loading)
            # matmul. These prologue transposes absorb the ident(Pool) and
            # wc_sb(DMA) waits so steady-state matmuls each carry <=1 wait.
            warm0 = pt_pool.tile([P, P], FP32, tag="pt")
            nc.tensor.transpose(warm0[:], ident[:], ident[:])
            warm1 = pt_pool.tile([P, P], FP32, tag="pt")
            nc.tensor.transpose(warm1[:], wc_sb[:, 0, :], ident[:])

            for tb in range(nblk):
                # x block: [128 tok partitions, SUB, G*128 feats] via ONE DMA
                x_sb = xin_pool.tile([P, SUB, G * P], FP32, tag="x_sb")
                nc.sync.dma_start(
                    out=x_sb[:],
                    in_=xin[ds(tb * BLK, BLK), :].rearrange(
                        "(j p) f -> p j f", p=P
                    ),
                )
                for g in range(G):
                    # 4 transposes -> 4 column slices of ONE psum bank
                    pt_big = pt_pool.tile([P, BLK], FP32, tag="pt")
                    for j in range(SUB):
                        # in_: [128 tok, 128 (kre|kim)] -> out: [128, 128 tok]
                        nc.tensor.transpose(
                            pt_big[:, ts(j, P)], x_sb[:, j, ds(P * g, P)], ident[:]
                        )
                    # single PSUM->SBUF copy per group (was 4)
                    xt_cat = xt_pool.tile([P, BLK], FP32, tag="xt_cat")
                    nc.vector.tensor_copy(out=xt_cat[:], in_=pt_big[:])
                    py = py_pool.tile([P, BLK], FP32, tag="py")
                    lhs = wc_sb[:, g, :]
                    rhs = xt_cat[:]
                    if f32r:
                        lhs = lhs.bitcast(F32R)
                        rhs = rhs.bitcast(F32R)
                    nc.tensor.matmul(py[:], lhs, rhs, start=True, stop=True)
                    y_sb = y_pool.tile([P, BLK], FP32, tag="y_sb")
                    nc.scalar.copy(out=y_sb[:], in_=py[:])
                    # stores ride the ACT HWDGE ring so a compute-blocked
                    # store can't head-of-line-block the next input load
                    # (which rides the SP ring)
                    nc.scalar.dma_start(
                        out=yt[g, :, ds(tb * BLK, BLK)], in_=y_sb[:]
                    )
    nc.compile()
    return nc


def _get_program(tok=TOK):
    f32r = bool(int(os.environ.get("KERNEL_F32R", "0")))
    key = (tok, f32r)
    if key not in _prog_cache:
        _prog_cache[key] = build_program(tok, f32r=f32r)
    return _prog_cache[key]


def _host_pack_w(weight):
    wre = np.ascontiguousarray(weight[..., 0], dtype=np.float32)  # [G,K,K]
    wim = np.ascontiguousarray(weight[..., 1], dtype=np.float32)
    wcat = np.empty((G, P, P), dtype=np.float32)
    wcat[:, :K, :K] = wre
    wcat[:, :K, K:] = wim
    wcat[:, K:, :K] = -wim
    wcat[:, K:, K:] = wre
    return wcat


# test.py can read the most recent BassKernelResults (exec_time_ns etc.)
last_results = None
last_trace_dir = None


def _install_ntff_hook():
    """The agent image's antenv lacks axon_hooks; register the NTFF profile
    hook ourselves so run_bass_kernel_spmd(trace=True) can time the kernel."""
    import sys
    import types

    if "antenv.axon_hooks" in sys.modules:
        return
    from trn_agent_boot.trn_boot import _ntff_profile_via_ctypes

    hook = _ntff_profile_via_ctypes("/opt/axon/libaxon_pjrt.so")
    mod = types.ModuleType("antenv.axon_hooks")
    mod.get_axon_ntff_profile_hook = lambda: hook
    sys.modules["antenv.axon_hooks"] = mod
    import antenv

    antenv.axon_hooks = mod
    import concourse.bass_utils as bu

    bu.upload_artifacts = lambda tmpdir: f"local://{tmpdir}"


def kernel(x_real, x_imag, weight):
    global last_results, last_trace_dir
    nc = _get_program()
    wcat = _host_pack_w(weight)
    msl = M // NCORES  # 512
    in_maps = []
    for c in range(NCORES):
        # interleave re/im per group: [tok, G, 2, K] contiguous
        xin_c = np.empty((TOK, G, 2, K), dtype=np.float32)
        xin_c[:, :, 0, :] = x_real[:, c * msl:(c + 1) * msl].reshape(TOK, G, K)
        xin_c[:, :, 1, :] = x_imag[:, c * msl:(c + 1) * msl].reshape(TOK, G, K)
        in_maps.append({"xin": xin_c.reshape(TOK, G * 2 * K), "wc": wcat})

    trace = bool(int(os.environ.get("KERNEL_TRACE", "0")))
    tmpdir = None
    if trace:
        import tempfile

        _install_ntff_hook()
        tmpdir = os.environ.get("KERNEL_TRACE_DIR") or tempfile.mkdtemp(
            prefix="bass_trace_"
        )
        last_trace_dir = tmpdir
    res = run_bass_kernel_spmd(
        nc, in_maps, list(range(NCORES)), trace=trace, tmpdir=tmpdir
    )
    last_results = res

    yt_all = np.stack([res.results[c]["yt"] for c in range(NCORES)])
    # [8, 32, 2(comp), 64(o), 4(b), 512(ml)] -> [b, c, ml, g, o, comp]
    y = (
        yt_all.reshape(NCORES, G, 2, K, B, msl)
        .transpose(4, 0, 5, 1, 3, 2)
        .reshape(B, M, G, K, 2)
    )
    return np.ascontiguousarray(y)
